# revision 10
# baseline (speedup 1.0000x reference)
"""Trainium2 Bass kernel for nn_MultiHeadAttention_56676388438432 (v4).

Reference math (all H=16 heads share identical weights, so they collapse):
    q = query @ Wq; k = key @ Wk; v = value @ Wv          (full-width, [B,S,D])
    qh = q @ wq_h + bq_h                                   ([B,S,64])
    scores = qh @ kh^T / 8, causal mask, softmax
    out_h = attn @ vh
    out = tile(out_h, 16) @ Wo

Algebraic collapse (exact):
    qh = query @ (Wq @ wq_h / 8) + bq_h/8   -> combined Aq [1024, 64]
    kh = key   @ (Wk @ wk_h) + bk_h         -> combined Ak
    vh = value @ (Wv @ wv_h)        (bv_h folded into Wo, see below)
    out = [out_h ; expsum] @ [WoS ; bv_h @ WoS]   (WoS = sum_h Wo[64h:64h+64])

The kernel is DMA-bound: 12.5 MB/core through an exclusive 360 B/ns DMA
device = 34.7 us floor. v4 restructures v2's schedule so the device never
idles and the post-stream tail is one 128-row chain instead of a 512-row
window:
  * window 1 (the deep 512 rows) is split into four 128-col AV chains in
    separate PSUM banks with staggered stop blocks (12, 13, 14, 15 at
    h=0 depth); their finishes and output DMAs spread through the
    stream instead of bunching after the last k/v byte.
  * k/v arrive one 128-row block at a time, ordered
    [8,0,9,1,10,2,11,3,12,4,5,6,7,13,14,15]: deep/shallow interleave
    balances per-arrival compute, every chain except the deepest quarter
    completes mid-stream, and the last three arrivals carry minimal work.
  * all DMAs ride the two HWDGE queues (SP + Act); outputs/esums go on
    Act, inputs on SP (the Pool SWDGE path costs ~2.4x per transfer).
  * scores are unpaired [128, <=512] per (window, block) so score PSUM
    tiles are single banks; exp widths shrink for blocks 13-15.

Sharding (unchanged from v2): 8 cores = 4 batches x 2 balanced causal
halves; w0 = rows [512h, 512h+512), w1 = rows [1536-512h, 2048-512h).
Each core redundantly computes kh/vh for its batch. Uniform SPMD
program; softmax division by expsum happens on host.
"""

import sys

sys.path.insert(0, "/opt/trn_rl_repo")

from contextlib import ExitStack

import numpy as np

import concourse.bass as bass
import concourse.tile as tile
from concourse import bacc
from concourse import mybir
from concourse.bass_utils import run_bass_kernel_spmd

B, S, D, H, HD = 4, 2048, 1024, 16, 64
P = 128
NCORES = 8

F32 = mybir.dt.float32
F32R = mybir.dt.float32r
BF16 = mybir.dt.bfloat16

W_SLOTS = (8, 16)
BAND = 1408  # mask band width: 128*7 + 512

WP_Q, WP_K, WP_V = 0, 512, 1024
WP_N = 1536

# k/v block arrival order: shallow/deep interleave; blocks 13-15 last so
# only the deepest quarter-chain's work trails the stream.
KV_ORDER = [8, 0, 9, 1, 10, 2, 11, 3, 12, 4, 5, 6, 7, 13, 14, 15]
# stop block per chain: w0 stops at 7; quarter k of w1 stops at 12+k,
# except quarter 0 whose block set {0..12} is exhausted at block 7 (the
# last of 4,5,6,7 in KV_ORDER).
Q_STOP = (7, 13, 14, 15)
W0_STOP = 7


def _q0(h, w):
    return 512 * h if w == 0 else 1536 - 512 * h


def _r(ap, dt):
    return ap.bitcast(dt)


def _emit(tc, io):
    """Emit the per-core program. io: dict of DRAM APs."""
    nc = tc.nc
    ctx = ExitStack()
    with ctx:
        # ---------------- pools ----------------
        const = ctx.enter_context(tc.tile_pool(name="const", bufs=1))
        atp = ctx.enter_context(tc.tile_pool(name="attn", bufs=6))
        ohp = ctx.enter_context(tc.tile_pool(name="oh", bufs=3))
        opool = ctx.enter_context(tc.tile_pool(name="o", bufs=4))

        ps_s = ctx.enter_context(tc.tile_pool(name="ps_s", bufs=2, space="PSUM"))
        ps_p = ctx.enter_context(tc.tile_pool(name="ps_p", bufs=1, space="PSUM"))
        ps_o = ctx.enter_context(tc.tile_pool(name="ps_o", bufs=5, space="PSUM"))

        # ---------------- resident SBUF tiles ----------------
        wp = const.tile([P, WP_N], BF16, tag="wp")
        wos = const.tile([65, D + 4], BF16, tag="wos")
        mu = const.tile([P, 2 * BAND], BF16, tag="mu")  # generated on device
        mg = const.tile([2, 128 + 2 * BAND], F32R, tag="mg")
        xq = const.tile([P, 8 * 1024], BF16, tag="xq")  # [p, (j, r)] j-chunk major
        # k and v share one tile: chunks 0..7 = k^T, 8..15 = v^T
        xkv = const.tile([P, 16 * S], BF16, tag="xkv")
        qh = const.tile([HD, 1024], F32R, tag="qh")
        kh = const.tile([HD, S], F32R, tag="kh")
        vh = const.tile([P, 16 * 65], BF16, tag="vh")  # [:, 65c:65c+64] + ones col

        # AV accumulators: w0 full bank; w1 quarters in their own banks
        # (one live accumulation chain per PSUM bank).
        pso_w0 = ps_o.tile([65, 512], F32, tag="po", name="pso_w0")
        pso_q = []
        for _qi in range(4):
            _pq = ps_o.tile([65, 512], F32, tag="po", name=f"pso_q{_qi}")
            pso_q.append(_pq)

        # ---------------- input DMAs (front-loaded, stream order) --------
        nc.gpsimd.memset(vh[:].rearrange("p (c e) -> p c e", e=65)[:, :, 64:65], 1.0)
        nc.scalar.dma_start(mg[:], _r(io["mgen"][:], F32R))
        nc.scalar.dma_start(wos[:], io["wos"][:])
        nc.sync.dma_start(wp[:], io["wp"][:])
        # w1's qh half first: the first k/v arrival is block 8 (w1-only)
        xq_s = xq[:].rearrange("p (j r) -> p j r", r=1024)
        xq_d = io["xqT"].rearrange("(j p) r -> p j r", p=P)
        nc.sync.dma_start(xq_s[:, :, 512:1024], xq_d[:, :, 512:1024])
        nc.sync.dma_start(xq_s[:, :, 0:512], xq_d[:, :, 0:512])
        xkv_s = xkv[:].rearrange("p (j r) -> p j r", r=S)
        xkv_d = io["xkv"].rearrange("(j p) r -> p j r", p=P)
        for c in KV_ORDER:
            nc.sync.dma_start(
                xkv_s[:, :, 128 * c : 128 * c + 128],
                xkv_d[:, :, 128 * c : 128 * c + 128],
            )

        # ---------------- device-side causal masks ----------------------
        # mu[p, w*BAND + u] = 1.0 iff p <= u - X_w, via a rank-2 PE outer
        # product (iota difference) and a DVE >=0 compare. Runs in the
        # DMA-bound opening microseconds and warms up the PE p-state.
        for w in range(2):
            for c0, cw in ((0, 512), (512, 512), (1024, BAND - 1024)):
                psm = ps_s.tile([P, 512], F32, tag="ps")
                for rep in range(2):
                    nc.tensor.matmul(
                        psm[:, 0:cw],
                        mg[:, 0:128],
                        mg[:, 128 + BAND * w + c0 : 128 + BAND * w + c0 + cw],
                        start=True,
                        stop=True,
                    )
                nc.vector.tensor_scalar(
                    mu[:, BAND * w + c0 : BAND * w + c0 + cw],
                    psm[:, 0:cw],
                    0.0,
                    None,
                    mybir.AluOpType.is_ge,
                )

        # ---------------- projections ----------------
        def proj_q(w):
            psp_full = ps_p.tile([P, 512], F32, tag="pp")
            psp = psp_full[0:HD]
            for j in range(8):
                nc.tensor.matmul(
                    psp,
                    wp[:, WP_Q + 64 * j : WP_Q + 64 * j + 64],
                    xq[:, 1024 * j + 512 * w : 1024 * j + 512 * w + 512],
                    start=(j == 0),
                    stop=(j == 7),
                )
            nc.scalar.activation(
                qh[:, 512 * w : 512 * w + 512],
                psp,
                mybir.ActivationFunctionType.Identity,
                bias=_r(wos[0:HD, D : D + 2], F32),
                scale=1.0,
            )

        def proj_kv(c):
            """kh and vh for k/v block c (rows [128c, 128c+128))."""
            psp_full = ps_p.tile([P, 512], F32, tag="pp")
            psk = psp_full[0:HD][:, 0:128]
            for j in range(8):
                nc.tensor.matmul(
                    psk,
                    wp[:, WP_K + 64 * j : WP_K + 64 * j + 64],
                    xkv[:, S * j + 128 * c : S * j + 128 * c + 128],
                    start=(j == 0),
                    stop=(j == 7),
                )
            nc.vector.tensor_scalar_add(
                kh[:, 128 * c : 128 * c + 128],
                psk,
                _r(wos[0:HD, D + 2 : D + 4], F32),
            )
            psv = psp_full[:, 256:320]
            for j in range(8):
                nc.tensor.matmul(
                    psv,
                    xkv[:, S * (8 + j) + 128 * c : S * (8 + j) + 128 * c + 128],
                    wp[:, WP_V + 64 * j : WP_V + 64 * j + 64],
                    start=(j == 0),
                    stop=(j == 7),
                )
            nc.vector.tensor_copy(vh[:, 65 * c : 65 * c + HD], psv)

        # ---------------- attention ----------------
        def attn_w(w, j):
            """Scores + exp (+ causal mask) of block j against window w.

            Returns the AV operand [128, 512] (cols = window q-columns;
            for trimmed deep blocks only the needed suffix is valid)."""
            if w == 0:
                lo = 0
            else:
                lo = 128 * max(0, j - 12)  # quarters k >= j-12 need block j
            width = 512 - lo
            pss = ps_s.tile([P, 512], F32, tag="ps")
            nc.tensor.matmul(
                pss[:, lo:512],
                kh[:, 128 * j : 128 * j + 128],
                qh[:, 512 * w + lo : 512 * w + 512],
                start=True,
                stop=True,
            )
            at = atp.tile([P, 512], BF16, tag="at")
            nc.scalar.activation(
                at[:, lo:512], pss[:, lo:512], mybir.ActivationFunctionType.Exp
            )
            # causal mask (multiplicative, on the exp'd tile). w0 bands
            # cover j<=7, w1 bands j>=8; other blocks are fully unmasked.
            toff = P * ((7 if w == 0 else 15) - j)
            if w == 0 or j >= 8:
                am = atp.tile([P, 512], BF16, tag="at")
                nc.vector.tensor_mul(
                    am[:, lo:512],
                    at[:, lo:512],
                    mu[:, BAND * w + toff + lo : BAND * w + toff + 512],
                )
                return am
            return at

        first = {"w0": True, "q": [True] * 4}

        def avs(j, src_w0, src_w1):
            if j < 8:
                nc.tensor.matmul(
                    pso_w0[:],
                    vh[:, 65 * j : 65 * j + 65],
                    src_w0[:, 0:512],
                    start=first["w0"],
                    stop=(j == W0_STOP),
                )
                first["w0"] = False
            for k in range(max(0, j - 12), 4):
                nc.tensor.matmul(
                    pso_q[k][:, 0:128],
                    vh[:, 65 * j : 65 * j + 65],
                    src_w1[:, 128 * k : 128 * k + 128],
                    start=first["q"][k],
                    stop=(j == Q_STOP[k]),
                )
                first["q"][k] = False

        def finish_w0():
            oh = ohp.tile([65, 512], BF16, tag="oh")
            nc.vector.tensor_copy(oh[:], pso_w0[:])
            nc.scalar.dma_start(io["esum"][0:1, :], oh[64:65, :])
            for t in range(4):
                ot = opool.tile([P, D], BF16, tag="o")
                pf = ps_s.tile([P, 512], F32, tag="ps")
                pf2 = ps_s.tile([P, 512], F32, tag="ps")
                nc.tensor.matmul(
                    pf, oh[:, 128 * t : 128 * t + 128], wos[:, 0:512],
                    start=True, stop=True,
                )
                nc.tensor.matmul(
                    pf2, oh[:, 128 * t : 128 * t + 128], wos[:, 512:1024],
                    start=True, stop=True,
                )
                nc.vector.tensor_copy(ot[:, 0:512], pf)
                nc.scalar.copy(ot[:, 512:1024], pf2)
                nc.scalar.dma_start(
                    io["out"][128 * t : 128 * t + 128, :], ot[:]
                )

        def finish_q(k):
            oh = ohp.tile([65, 512], BF16, tag="oh")
            nc.vector.tensor_copy(oh[:, 0:128], pso_q[k][:, 0:128])
            nc.scalar.dma_start(
                io["esum"][1:2, 128 * k : 128 * k + 128], oh[64:65, 0:128]
            )
            ot = opool.tile([P, D], BF16, tag="o")
            pf = ps_s.tile([P, 512], F32, tag="ps")
            pf2 = ps_s.tile([P, 512], F32, tag="ps")
            nc.tensor.matmul(pf, oh[:, 0:128], wos[:, 0:512], start=True, stop=True)
            nc.tensor.matmul(
                pf2, oh[:, 0:128], wos[:, 512:1024], start=True, stop=True
            )
            if k % 2 == 0:
                nc.vector.tensor_copy(ot[:, 0:512], pf)
                nc.scalar.copy(ot[:, 512:1024], pf2)
            else:
                nc.scalar.copy(ot[:, 0:512], pf)
                nc.vector.tensor_copy(ot[:, 512:1024], pf2)
            nc.scalar.dma_start(
                io["out"][512 + 128 * k : 512 + 128 * k + 128, :], ot[:]
            )

        # ---------------- schedule ----------------
        proj_q(1)
        proj_q(0)
        for c in KV_ORDER:
            proj_kv(c)
            a1 = attn_w(1, c)
            a0 = attn_w(0, c) if c < 8 else None
            avs(c, a0, a1)
            if c == W0_STOP:
                finish_w0()
                finish_q(0)
            for k in (1, 2, 3):
                if c == Q_STOP[k]:
                    finish_q(k)


_CACHE = {}


def _build():
    if "nc" in _CACHE:
        return _CACHE["nc"]
    nc = bacc.Bacc("TRN2", target_bir_lowering=False, debug=False, num_devices=NCORES)
    io = {}
    io["xqT"] = nc.dram_tensor("xqT", [D, 1024], BF16, kind="ExternalInput").ap()
    io["xkv"] = nc.dram_tensor("xkv", [2 * D, S], BF16, kind="ExternalInput").ap()
    io["wp"] = nc.dram_tensor("wp", [P, WP_N], BF16, kind="ExternalInput").ap()
    io["wos"] = nc.dram_tensor("wos", [65, D + 4], BF16, kind="ExternalInput").ap()
    io["mgen"] = nc.dram_tensor("mgen", [2, 128 + 2 * BAND], F32, kind="ExternalInput").ap()
    io["out"] = nc.dram_tensor("out", [1024, D], BF16, kind="ExternalOutput").ap()
    io["esum"] = nc.dram_tensor("esum", [2, 512], BF16, kind="ExternalOutput").ap()
    with tile.TileContext(nc) as tc:
        _emit(tc, io)
    nc.compile()
    _CACHE["nc"] = nc
    return nc


def _host_prep(query, key, value, mask, Wq, Wk, Wv, wq_h, bq_h, wk_h, bk_h, wv_h,
               bv_h, Wo):
    """Combine weights on host (exact algebra, float64 accumulate)."""
    Aq = (np.asarray(Wq, np.float64) @ np.asarray(wq_h, np.float64) / 8.0).astype(
        np.float32
    )
    Ak = (np.asarray(Wk, np.float64) @ np.asarray(wk_h, np.float64)).astype(np.float32)
    Av = (np.asarray(Wv, np.float64) @ np.asarray(wv_h, np.float64)).astype(np.float32)
    bq = (np.asarray(bq_h, np.float64) / 8.0).astype(np.float32)
    bk = np.asarray(bk_h, np.float32)
    WoS = np.asarray(Wo, np.float64).reshape(H, HD, D).sum(axis=0)
    wos_aug = np.concatenate(
        [WoS, (np.asarray(bv_h, np.float64) @ WoS)[None, :]], axis=0
    ).astype(np.float32)
    # biases ride as raw f32 bits in two bf16-pair columns:
    # cols D:D+2 = bq, cols D+2:D+4 = bk (device bitcasts back to f32)
    import ml_dtypes
    wos_ext = np.zeros((65, D + 4), ml_dtypes.bfloat16)
    wos_ext[:, 0:D] = wos_aug.astype(ml_dtypes.bfloat16)
    u16 = wos_ext.view(np.uint16)
    u16[0:HD, D : D + 2] = bq.astype(np.float32).view(np.uint16).reshape(HD, 2)
    u16[0:HD, D + 2 : D + 4] = bk.astype(np.float32).view(np.uint16).reshape(HD, 2)
    return Aq, Ak, Av, wos_ext


def _pack_w(A):
    """[1024, 64] -> [128, 512] partition-packed layout."""
    return np.ascontiguousarray(
        A.reshape(8, P, HD).transpose(1, 0, 2).reshape(P, 512)
    )


def _mk_mgen(h):
    """Inputs for device-side mask generation.

    Row 0: [ones(128) | u - X_0 | u - X_1]; row 1: [iota(128) | -1 | -1].
    The PE computes psm[p, u] = (u - X_w) - p; keep iff >= 0.
    """
    u = np.arange(BAND, dtype=np.float32)
    mgen = np.empty((2, 128 + 2 * BAND), np.float32)
    mgen[0, 0:128] = 1.0
    mgen[1, 0:128] = np.arange(128, dtype=np.float32)
    mgen[1, 128:] = -1.0
    mgen[0, 128 : 128 + BAND] = u - np.float32(896 - _q0(h, 0))
    mgen[0, 128 + BAND :] = u - np.float32(1920 - _q0(h, 1))
    return mgen


def _numpy_fallback(query, key, value, mask, Wq, Wk, Wv, wq_h, bq_h, wk_h, bk_h,
                    wv_h, bv_h, Wo):
    q = query @ Wq
    k = key @ Wk
    v = value @ Wv
    qh = q @ wq_h + bq_h
    kh = k @ wk_h + bk_h
    vh = v @ wv_h + bv_h
    scores = np.einsum("bsh,bth->bst", qh, kh) / np.sqrt(np.float32(HD))
    scores = np.where(mask, np.float32(-1e9), scores)
    scores = scores - scores.max(axis=-1, keepdims=True)
    e = np.exp(scores)
    attn = e / e.sum(axis=-1, keepdims=True)
    out_h = np.einsum("bst,bth->bsh", attn, vh)
    out = np.tile(out_h, (1, 1, H))
    return (out @ Wo).astype(np.float32)


def kernel(**inputs):
    import ml_dtypes

    inputs = {k: np.asarray(v) for k, v in inputs.items()}
    mask = inputs["mask"]
    causal = np.array_equal(mask, np.triu(np.ones((S, S), bool), k=1))
    if not causal:
        return _numpy_fallback(**inputs)

    query, key, value = inputs["query"], inputs["key"], inputs["value"]
    Aq, Ak, Av, wos_ext = _host_prep(**inputs)

    wp = np.zeros((P, WP_N), ml_dtypes.bfloat16)
    wp[:, WP_Q : WP_Q + 512] = _pack_w(Aq).astype(ml_dtypes.bfloat16)
    wp[:, WP_K : WP_K + 512] = _pack_w(Ak).astype(ml_dtypes.bfloat16)
    wp[:, WP_V : WP_V + 512] = _pack_w(Av).astype(ml_dtypes.bfloat16)

    nc = _build()
    xkv = {}
    for b in range(B):
        buf = np.empty((2 * D, S), ml_dtypes.bfloat16)
        buf[0:D] = key[b].T.astype(ml_dtypes.bfloat16)
        buf[D:] = value[b].T.astype(ml_dtypes.bfloat16)
        xkv[b] = buf
    in_maps = []
    for c in range(NCORES):
        b, h = c // 2, c % 2
        xq_rows = np.concatenate(
            [
                query[b, _q0(h, 0) : _q0(h, 0) + 512],
                query[b, _q0(h, 1) : _q0(h, 1) + 512],
            ],
            axis=0,
        )
        in_maps.append(
            {
                "xqT": np.ascontiguousarray(xq_rows.T.astype(ml_dtypes.bfloat16)),
                "xkv": xkv[b],
                "wp": wp,
                "wos": wos_ext,
                "mgen": _mk_mgen(h),
            }
        )

    res = run_bass_kernel_spmd(nc, in_maps, list(range(NCORES)))
    out = np.empty((B, S, D), np.float32)
    for c in range(NCORES):
        b, h = c // 2, c % 2
        co = np.asarray(res.results[c]["out"]).astype(np.float32)
        es = np.asarray(res.results[c]["esum"]).astype(np.float32)
        co[0:512] /= es[0][:, None]
        co[512:1024] /= es[1][:, None]
        out[b, _q0(h, 0) : _q0(h, 0) + 512] = co[0:512]
        out[b, _q0(h, 1) : _q0(h, 1) + 512] = co[512:1024]
    return out


if __name__ == "__main__":
    nc = _build()
    print("build ok")


# revision 13
# speedup vs baseline: 1.1879x; 1.1879x over previous
"""Trainium2 Bass kernel for nn_MultiHeadAttention_56676388438432 (v4).

Reference math (all H=16 heads share identical weights, so they collapse):
    q = query @ Wq; k = key @ Wk; v = value @ Wv          (full-width, [B,S,D])
    qh = q @ wq_h + bq_h                                   ([B,S,64])
    scores = qh @ kh^T / 8, causal mask, softmax
    out_h = attn @ vh
    out = tile(out_h, 16) @ Wo

Algebraic collapse (exact):
    qh = query @ (Wq @ wq_h / 8) + bq_h/8   -> combined Aq [1024, 64]
    kh = key   @ (Wk @ wk_h) + bk_h         -> combined Ak
    vh = value @ (Wv @ wv_h)        (bv_h folded into Wo, see below)
    out = [out_h ; expsum] @ [WoS ; bv_h @ WoS]   (WoS = sum_h Wo[64h:64h+64])

The kernel is DMA-bound: 12.5 MB/core through an exclusive 360 B/ns DMA
device = 34.7 us floor. v4 restructures v2's schedule so the device never
idles and the post-stream tail is one 128-row chain instead of a 512-row
window:
  * window 1 (the deep 512 rows) is split into four 128-col AV chains in
    separate PSUM banks with staggered stop blocks (12, 13, 14, 15 at
    h=0 depth); their finishes and output DMAs spread through the
    stream instead of bunching after the last k/v byte.
  * k/v arrive one 128-row block at a time, ordered
    [8,0,9,1,10,2,11,3,12,4,5,6,7,13,14,15]: deep/shallow interleave
    balances per-arrival compute, every chain except the deepest quarter
    completes mid-stream, and the last three arrivals carry minimal work.
  * all DMAs ride the two HWDGE queues (SP + Act); outputs/esums go on
    Act, inputs on SP (the Pool SWDGE path costs ~2.4x per transfer).
  * scores are unpaired [128, <=512] per (window, block) so score PSUM
    tiles are single banks; exp widths shrink for blocks 13-15.

Sharding (unchanged from v2): 8 cores = 4 batches x 2 balanced causal
halves; w0 = rows [512h, 512h+512), w1 = rows [1536-512h, 2048-512h).
Each core redundantly computes kh/vh for its batch. Uniform SPMD
program; softmax division by expsum happens on host.
"""

import sys

sys.path.insert(0, "/opt/trn_rl_repo")

from contextlib import ExitStack

import numpy as np

import concourse.bass as bass
import concourse.tile as tile
from concourse import bacc
from concourse import mybir
from concourse.bass_utils import run_bass_kernel_spmd

B, S, D, H, HD = 4, 2048, 1024, 16, 64
P = 128
NCORES = 8

F32 = mybir.dt.float32
F32R = mybir.dt.float32r
BF16 = mybir.dt.bfloat16

W_SLOTS = (8, 16)
BAND = 1408  # mask band width: 128*7 + 512

WP_Q, WP_K, WP_V = 0, 512, 1024
WP_N = 1536

# k/v block arrival order: shallow/deep interleave; blocks 13-15 last so
# only the deepest quarter-chain's work trails the stream.
KV_ORDER = [8, 0, 9, 1, 10, 2, 11, 3, 12, 4, 5, 6, 7, 13, 14, 15]
# stop block per chain: w0 stops at 7; quarter k of w1 stops at 12+k,
# except quarter 0 whose block set {0..12} is exhausted at block 7 (the
# last of 4,5,6,7 in KV_ORDER).
Q_STOP = (7, 13, 14, 15)
W0_STOP = 7


def _q0(h, w):
    return 512 * h if w == 0 else 1536 - 512 * h


def _r(ap, dt):
    return ap.bitcast(dt)


def _emit(tc, io):
    """Emit the per-core program. io: dict of DRAM APs."""
    nc = tc.nc
    ctx = ExitStack()
    with ctx:
        # ---------------- pools ----------------
        const = ctx.enter_context(tc.tile_pool(name="const", bufs=1))
        atp = ctx.enter_context(tc.tile_pool(name="attn", bufs=6))
        ohp = ctx.enter_context(tc.tile_pool(name="oh", bufs=3))
        opool = ctx.enter_context(tc.tile_pool(name="o", bufs=4))

        ps_s = ctx.enter_context(tc.tile_pool(name="ps_s", bufs=2, space="PSUM"))
        ps_p = ctx.enter_context(tc.tile_pool(name="ps_p", bufs=1, space="PSUM"))
        ps_o = ctx.enter_context(tc.tile_pool(name="ps_o", bufs=5, space="PSUM"))

        # ---------------- resident SBUF tiles ----------------
        wp = const.tile([P, WP_N], BF16, tag="wp")
        wos = const.tile([65, D + 4], BF16, tag="wos")
        mu = const.tile([P, 2 * BAND], BF16, tag="mu")  # generated on device
        mg = const.tile([2, 128 + 2 * BAND], F32R, tag="mg")
        xq = const.tile([P, 8 * 1024], BF16, tag="xq")  # [p, (j, r)] j-chunk major
        # k and v share one tile: chunks 0..7 = k^T, 8..15 = v^T
        xkv = const.tile([P, 16 * S], BF16, tag="xkv")
        qh = const.tile([HD, 1024], F32R, tag="qh")
        kh = const.tile([HD, S], F32R, tag="kh")
        vh = const.tile([P, 16 * 65], BF16, tag="vh")  # [:, 65c:65c+64] + ones col

        # AV accumulators: w0 full bank; w1 quarters in their own banks
        # (one live accumulation chain per PSUM bank).
        pso_w0 = ps_o.tile([65, 512], F32, tag="po", name="pso_w0")
        pso_q = []
        for _qi in range(4):
            _pq = ps_o.tile([65, 512], F32, tag="po", name=f"pso_q{_qi}")
            pso_q.append(_pq)

        # ---------------- input DMAs (front-loaded, stream order) --------
        nc.gpsimd.memset(vh[:].rearrange("p (c e) -> p c e", e=65)[:, :, 64:65], 1.0)
        nc.scalar.dma_start(mg[:], _r(io["mgen"][:], F32R))
        nc.scalar.dma_start(wos[:], io["wos"][:])
        nc.sync.dma_start(wp[:], io["wp"][:])
        # w1's qh half first: the first k/v arrival is block 8 (w1-only)
        xq_s = xq[:].rearrange("p (j r) -> p j r", r=1024)
        xq_d = io["xqT"].rearrange("(j p) r -> p j r", p=P)
        nc.sync.dma_start(xq_s[:, :, 512:1024], xq_d[:, :, 512:1024])
        nc.sync.dma_start(xq_s[:, :, 0:512], xq_d[:, :, 0:512])
        # xkv columns are host-reordered into KV_ORDER so each arrival piece
        # (2 blocks = 1 MB) is contiguous with 512 B runs (no small-elem
        # penalty). SBUF keeps the reordered layout; _slot maps global block
        # id -> reordered slot.
        xkv_s = xkv[:].rearrange("p (j r) -> p j r", r=S)
        xkv_d = io["xkv"].rearrange("(j p) r -> p j r", p=P)
        for s in range(8):
            nc.sync.dma_start(
                xkv_s[:, :, 256 * s : 256 * s + 256],
                xkv_d[:, :, 256 * s : 256 * s + 256],
            )

        # ---------------- device-side causal masks ----------------------
        # mu[p, w*BAND + u] = 1.0 iff p <= u - X_w, via a rank-2 PE outer
        # product (iota difference) and a DVE >=0 compare. Runs in the
        # DMA-bound opening microseconds and warms up the PE p-state.
        for w in range(2):
            for c0, cw in ((0, 512), (512, 512), (1024, BAND - 1024)):
                psm = ps_s.tile([P, 512], F32, tag="ps")
                for rep in range(2):
                    nc.tensor.matmul(
                        psm[:, 0:cw],
                        mg[:, 0:128],
                        mg[:, 128 + BAND * w + c0 : 128 + BAND * w + c0 + cw],
                        start=True,
                        stop=True,
                    )
                nc.vector.tensor_scalar(
                    mu[:, BAND * w + c0 : BAND * w + c0 + cw],
                    psm[:, 0:cw],
                    0.0,
                    None,
                    mybir.AluOpType.is_ge,
                )

        # ---------------- projections ----------------
        def proj_q(w):
            psp_full = ps_p.tile([P, 512], F32, tag="pp")
            psp = psp_full[0:HD]
            for j in range(8):
                nc.tensor.matmul(
                    psp,
                    wp[:, WP_Q + 64 * j : WP_Q + 64 * j + 64],
                    xq[:, 1024 * j + 512 * w : 1024 * j + 512 * w + 512],
                    start=(j == 0),
                    stop=(j == 7),
                )
            nc.scalar.activation(
                qh[:, 512 * w : 512 * w + 512],
                psp,
                mybir.ActivationFunctionType.Identity,
                bias=_r(wos[0:HD, D : D + 2], F32),
                scale=1.0,
            )

        def proj_kv(c):
            """kh and vh for k/v block c (rows [128c, 128c+128))."""
            s = KV_ORDER.index(c)  # reordered SBUF slot
            psp_full = ps_p.tile([P, 512], F32, tag="pp")
            psk = psp_full[0:HD][:, 0:128]
            for j in range(8):
                nc.tensor.matmul(
                    psk,
                    wp[:, WP_K + 64 * j : WP_K + 64 * j + 64],
                    xkv[:, S * j + 128 * s : S * j + 128 * s + 128],
                    start=(j == 0),
                    stop=(j == 7),
                )
            nc.vector.tensor_scalar_add(
                kh[:, 128 * c : 128 * c + 128],
                psk,
                _r(wos[0:HD, D + 2 : D + 4], F32),
            )
            psv = psp_full[:, 256:320]
            for j in range(8):
                nc.tensor.matmul(
                    psv,
                    xkv[:, S * (8 + j) + 128 * s : S * (8 + j) + 128 * s + 128],
                    wp[:, WP_V + 64 * j : WP_V + 64 * j + 64],
                    start=(j == 0),
                    stop=(j == 7),
                )
            nc.vector.tensor_copy(vh[:, 65 * c : 65 * c + HD], psv)

        # ---------------- attention ----------------
        def attn_w(w, j):
            """Scores + exp (+ causal mask) of block j against window w.

            Returns the AV operand [128, 512] (cols = window q-columns;
            for trimmed deep blocks only the needed suffix is valid)."""
            if w == 0:
                lo = 0
            else:
                lo = 128 * max(0, j - 12)  # quarters k >= j-12 need block j
            width = 512 - lo
            pss = ps_s.tile([P, 512], F32, tag="ps")
            nc.tensor.matmul(
                pss[:, lo:512],
                kh[:, 128 * j : 128 * j + 128],
                qh[:, 512 * w + lo : 512 * w + 512],
                start=True,
                stop=True,
            )
            at = atp.tile([P, 512], BF16, tag="at")
            nc.scalar.activation(
                at[:, lo:512], pss[:, lo:512], mybir.ActivationFunctionType.Exp
            )
            # causal mask (multiplicative, on the exp'd tile). w0 bands
            # cover j<=7, w1 bands j>=8; other blocks are fully unmasked.
            toff = P * ((7 if w == 0 else 15) - j)
            if w == 0 or j >= 8:
                am = atp.tile([P, 512], BF16, tag="at")
                nc.vector.tensor_mul(
                    am[:, lo:512],
                    at[:, lo:512],
                    mu[:, BAND * w + toff + lo : BAND * w + toff + 512],
                )
                return am
            return at

        first = {"w0": True, "q": [True] * 4}

        def avs(j, src_w0, src_w1):
            if j < 8:
                nc.tensor.matmul(
                    pso_w0[:],
                    vh[:, 65 * j : 65 * j + 65],
                    src_w0[:, 0:512],
                    start=first["w0"],
                    stop=(j == W0_STOP),
                )
                first["w0"] = False
            for k in range(max(0, j - 12), 4):
                nc.tensor.matmul(
                    pso_q[k][:, 0:128],
                    vh[:, 65 * j : 65 * j + 65],
                    src_w1[:, 128 * k : 128 * k + 128],
                    start=first["q"][k],
                    stop=(j == Q_STOP[k]),
                )
                first["q"][k] = False

        def finish_w0():
            oh = ohp.tile([65, 512], BF16, tag="oh")
            nc.vector.tensor_copy(oh[:], pso_w0[:])
            nc.scalar.dma_start(io["esum"][0:1, :], oh[64:65, :])
            for t in range(4):
                ot = opool.tile([P, D], BF16, tag="o")
                pf = ps_s.tile([P, 512], F32, tag="ps")
                pf2 = ps_s.tile([P, 512], F32, tag="ps")
                nc.tensor.matmul(
                    pf, oh[:, 128 * t : 128 * t + 128], wos[:, 0:512],
                    start=True, stop=True,
                )
                nc.tensor.matmul(
                    pf2, oh[:, 128 * t : 128 * t + 128], wos[:, 512:1024],
                    start=True, stop=True,
                )
                nc.vector.tensor_copy(ot[:, 0:512], pf)
                nc.scalar.copy(ot[:, 512:1024], pf2)
                nc.scalar.dma_start(
                    io["out"][128 * t : 128 * t + 128, :], ot[:]
                )

        def finish_q(k):
            oh = ohp.tile([65, 512], BF16, tag="oh")
            nc.vector.tensor_copy(oh[:, 0:128], pso_q[k][:, 0:128])
            nc.scalar.dma_start(
                io["esum"][1:2, 128 * k : 128 * k + 128], oh[64:65, 0:128]
            )
            ot = opool.tile([P, D], BF16, tag="o")
            pf = ps_s.tile([P, 512], F32, tag="ps")
            pf2 = ps_s.tile([P, 512], F32, tag="ps")
            nc.tensor.matmul(pf, oh[:, 0:128], wos[:, 0:512], start=True, stop=True)
            nc.tensor.matmul(
                pf2, oh[:, 0:128], wos[:, 512:1024], start=True, stop=True
            )
            if k % 2 == 0:
                nc.vector.tensor_copy(ot[:, 0:512], pf)
                nc.scalar.copy(ot[:, 512:1024], pf2)
            else:
                nc.scalar.copy(ot[:, 0:512], pf)
                nc.vector.tensor_copy(ot[:, 512:1024], pf2)
            nc.scalar.dma_start(
                io["out"][512 + 128 * k : 512 + 128 * k + 128, :], ot[:]
            )

        # ---------------- schedule ----------------
        proj_q(1)
        proj_q(0)
        for c in KV_ORDER:
            proj_kv(c)
            a1 = attn_w(1, c)
            a0 = attn_w(0, c) if c < 8 else None
            avs(c, a0, a1)
            if c == W0_STOP:
                finish_w0()
                finish_q(0)
            for k in (1, 2, 3):
                if c == Q_STOP[k]:
                    finish_q(k)


_CACHE = {}


def _build():
    if "nc" in _CACHE:
        return _CACHE["nc"]
    nc = bacc.Bacc("TRN2", target_bir_lowering=False, debug=False, num_devices=NCORES)
    io = {}
    io["xqT"] = nc.dram_tensor("xqT", [D, 1024], BF16, kind="ExternalInput").ap()
    io["xkv"] = nc.dram_tensor("xkv", [2 * D, S], BF16, kind="ExternalInput").ap()
    io["wp"] = nc.dram_tensor("wp", [P, WP_N], BF16, kind="ExternalInput").ap()
    io["wos"] = nc.dram_tensor("wos", [65, D + 4], BF16, kind="ExternalInput").ap()
    io["mgen"] = nc.dram_tensor("mgen", [2, 128 + 2 * BAND], F32, kind="ExternalInput").ap()
    io["out"] = nc.dram_tensor("out", [1024, D], BF16, kind="ExternalOutput").ap()
    io["esum"] = nc.dram_tensor("esum", [2, 512], BF16, kind="ExternalOutput").ap()
    with tile.TileContext(nc) as tc:
        _emit(tc, io)
    nc.compile()
    _CACHE["nc"] = nc
    return nc


def _host_prep(query, key, value, mask, Wq, Wk, Wv, wq_h, bq_h, wk_h, bk_h, wv_h,
               bv_h, Wo):
    """Combine weights on host (exact algebra, float64 accumulate)."""
    Aq = (np.asarray(Wq, np.float64) @ np.asarray(wq_h, np.float64) / 8.0).astype(
        np.float32
    )
    Ak = (np.asarray(Wk, np.float64) @ np.asarray(wk_h, np.float64)).astype(np.float32)
    Av = (np.asarray(Wv, np.float64) @ np.asarray(wv_h, np.float64)).astype(np.float32)
    bq = (np.asarray(bq_h, np.float64) / 8.0).astype(np.float32)
    bk = np.asarray(bk_h, np.float32)
    WoS = np.asarray(Wo, np.float64).reshape(H, HD, D).sum(axis=0)
    wos_aug = np.concatenate(
        [WoS, (np.asarray(bv_h, np.float64) @ WoS)[None, :]], axis=0
    ).astype(np.float32)
    # biases ride as raw f32 bits in two bf16-pair columns:
    # cols D:D+2 = bq, cols D+2:D+4 = bk (device bitcasts back to f32)
    import ml_dtypes
    wos_ext = np.zeros((65, D + 4), ml_dtypes.bfloat16)
    wos_ext[:, 0:D] = wos_aug.astype(ml_dtypes.bfloat16)
    u16 = wos_ext.view(np.uint16)
    u16[0:HD, D : D + 2] = bq.astype(np.float32).view(np.uint16).reshape(HD, 2)
    u16[0:HD, D + 2 : D + 4] = bk.astype(np.float32).view(np.uint16).reshape(HD, 2)
    return Aq, Ak, Av, wos_ext


def _pack_w(A):
    """[1024, 64] -> [128, 512] partition-packed layout."""
    return np.ascontiguousarray(
        A.reshape(8, P, HD).transpose(1, 0, 2).reshape(P, 512)
    )


def _mk_mgen(h):
    """Inputs for device-side mask generation.

    Row 0: [ones(128) | u - X_0 | u - X_1]; row 1: [iota(128) | -1 | -1].
    The PE computes psm[p, u] = (u - X_w) - p; keep iff >= 0.
    """
    u = np.arange(BAND, dtype=np.float32)
    mgen = np.empty((2, 128 + 2 * BAND), np.float32)
    mgen[0, 0:128] = 1.0
    mgen[1, 0:128] = np.arange(128, dtype=np.float32)
    mgen[1, 128:] = -1.0
    mgen[0, 128 : 128 + BAND] = u - np.float32(896 - _q0(h, 0))
    mgen[0, 128 + BAND :] = u - np.float32(1920 - _q0(h, 1))
    return mgen


def _numpy_fallback(query, key, value, mask, Wq, Wk, Wv, wq_h, bq_h, wk_h, bk_h,
                    wv_h, bv_h, Wo):
    q = query @ Wq
    k = key @ Wk
    v = value @ Wv
    qh = q @ wq_h + bq_h
    kh = k @ wk_h + bk_h
    vh = v @ wv_h + bv_h
    scores = np.einsum("bsh,bth->bst", qh, kh) / np.sqrt(np.float32(HD))
    scores = np.where(mask, np.float32(-1e9), scores)
    scores = scores - scores.max(axis=-1, keepdims=True)
    e = np.exp(scores)
    attn = e / e.sum(axis=-1, keepdims=True)
    out_h = np.einsum("bst,bth->bsh", attn, vh)
    out = np.tile(out_h, (1, 1, H))
    return (out @ Wo).astype(np.float32)


def kernel(**inputs):
    import ml_dtypes

    inputs = {k: np.asarray(v) for k, v in inputs.items()}
    mask = inputs["mask"]
    causal = np.array_equal(mask, np.triu(np.ones((S, S), bool), k=1))
    if not causal:
        return _numpy_fallback(**inputs)

    query, key, value = inputs["query"], inputs["key"], inputs["value"]
    Aq, Ak, Av, wos_ext = _host_prep(**inputs)

    wp = np.zeros((P, WP_N), ml_dtypes.bfloat16)
    wp[:, WP_Q : WP_Q + 512] = _pack_w(Aq).astype(ml_dtypes.bfloat16)
    wp[:, WP_K : WP_K + 512] = _pack_w(Ak).astype(ml_dtypes.bfloat16)
    wp[:, WP_V : WP_V + 512] = _pack_w(Av).astype(ml_dtypes.bfloat16)

    nc = _build()
    xkv = {}
    for b in range(B):
        kT = key[b].T.astype(ml_dtypes.bfloat16)
        vT = value[b].T.astype(ml_dtypes.bfloat16)
        buf = np.empty((2 * D, S), ml_dtypes.bfloat16)
        # columns reordered into arrival order (KV_ORDER): slot s holds
        # global 128-row block KV_ORDER[s]
        for s, c in enumerate(KV_ORDER):
            buf[0:D, 128 * s : 128 * s + 128] = kT[:, 128 * c : 128 * c + 128]
            buf[D:, 128 * s : 128 * s + 128] = vT[:, 128 * c : 128 * c + 128]
        xkv[b] = buf
    in_maps = []
    for c in range(NCORES):
        b, h = c // 2, c % 2
        xq_rows = np.concatenate(
            [
                query[b, _q0(h, 0) : _q0(h, 0) + 512],
                query[b, _q0(h, 1) : _q0(h, 1) + 512],
            ],
            axis=0,
        )
        in_maps.append(
            {
                "xqT": np.ascontiguousarray(xq_rows.T.astype(ml_dtypes.bfloat16)),
                "xkv": xkv[b],
                "wp": wp,
                "wos": wos_ext,
                "mgen": _mk_mgen(h),
            }
        )

    res = run_bass_kernel_spmd(nc, in_maps, list(range(NCORES)))
    out = np.empty((B, S, D), np.float32)
    for c in range(NCORES):
        b, h = c // 2, c % 2
        co = np.asarray(res.results[c]["out"]).astype(np.float32)
        es = np.asarray(res.results[c]["esum"]).astype(np.float32)
        co[0:512] /= es[0][:, None]
        co[512:1024] /= es[1][:, None]
        out[b, _q0(h, 0) : _q0(h, 0) + 512] = co[0:512]
        out[b, _q0(h, 1) : _q0(h, 1) + 512] = co[512:1024]
    return out


if __name__ == "__main__":
    nc = _build()
    print("build ok")


# revision 14
# speedup vs baseline: 1.2745x; 1.0729x over previous
"""Trainium2 Bass kernel for nn_MultiHeadAttention_56676388438432 (v4).

Reference math (all H=16 heads share identical weights, so they collapse):
    q = query @ Wq; k = key @ Wk; v = value @ Wv          (full-width, [B,S,D])
    qh = q @ wq_h + bq_h                                   ([B,S,64])
    scores = qh @ kh^T / 8, causal mask, softmax
    out_h = attn @ vh
    out = tile(out_h, 16) @ Wo

Algebraic collapse (exact):
    qh = query @ (Wq @ wq_h / 8) + bq_h/8   -> combined Aq [1024, 64]
    kh = key   @ (Wk @ wk_h) + bk_h         -> combined Ak
    vh = value @ (Wv @ wv_h)        (bv_h folded into Wo, see below)
    out = [out_h ; expsum] @ [WoS ; bv_h @ WoS]   (WoS = sum_h Wo[64h:64h+64])

The kernel is DMA-bound: 12.5 MB/core through an exclusive 360 B/ns DMA
device = 34.7 us floor. v4 restructures v2's schedule so the device never
idles and the post-stream tail is one 128-row chain instead of a 512-row
window:
  * window 1 (the deep 512 rows) is split into four 128-col AV chains in
    separate PSUM banks with staggered stop blocks (12, 13, 14, 15 at
    h=0 depth); their finishes and output DMAs spread through the
    stream instead of bunching after the last k/v byte.
  * k/v arrive one 128-row block at a time, ordered
    [8,0,9,1,10,2,11,3,12,4,5,6,7,13,14,15]: deep/shallow interleave
    balances per-arrival compute, every chain except the deepest quarter
    completes mid-stream, and the last three arrivals carry minimal work.
  * all DMAs ride the two HWDGE queues (SP + Act); outputs/esums go on
    Act, inputs on SP (the Pool SWDGE path costs ~2.4x per transfer).
  * scores are unpaired [128, <=512] per (window, block) so score PSUM
    tiles are single banks; exp widths shrink for blocks 13-15.

Sharding (unchanged from v2): 8 cores = 4 batches x 2 balanced causal
halves; w0 = rows [512h, 512h+512), w1 = rows [1536-512h, 2048-512h).
Each core redundantly computes kh/vh for its batch. Uniform SPMD
program; softmax division by expsum happens on host.
"""

import sys

sys.path.insert(0, "/opt/trn_rl_repo")

from contextlib import ExitStack

import numpy as np

import concourse.bass as bass
import concourse.tile as tile
from concourse import bacc
from concourse import mybir
from concourse.bass_utils import run_bass_kernel_spmd

B, S, D, H, HD = 4, 2048, 1024, 16, 64
P = 128
NCORES = 8

F32 = mybir.dt.float32
F32R = mybir.dt.float32r
BF16 = mybir.dt.bfloat16

W_SLOTS = (8, 16)
BAND = 1408  # mask band width: 128*7 + 512

WP_Q, WP_K, WP_V = 0, 512, 1024
WP_N = 1536

# k/v block arrival order: shallow/deep interleave; blocks 13-15 last so
# only the deepest quarter-chain's work trails the stream.
KV_ORDER = [8, 0, 9, 1, 10, 2, 11, 3, 12, 4, 5, 6, 7, 13, 14, 15]
# stop block per chain: w0 stops at 7; quarter k of w1 stops at 12+k,
# except quarter 0 whose block set {0..12} is exhausted at block 7 (the
# last of 4,5,6,7 in KV_ORDER).
Q_STOP = (7, 13, 14, 15)
W0_STOP = 7


def _q0(h, w):
    return 512 * h if w == 0 else 1536 - 512 * h


def _r(ap, dt):
    return ap.bitcast(dt)


def _emit(tc, io):
    """Emit the per-core program. io: dict of DRAM APs."""
    nc = tc.nc
    ctx = ExitStack()
    with ctx:
        # ---------------- pools ----------------
        const = ctx.enter_context(tc.tile_pool(name="const", bufs=1))
        atp = ctx.enter_context(tc.tile_pool(name="attn", bufs=6))
        ohp = ctx.enter_context(tc.tile_pool(name="oh", bufs=3))
        opool = ctx.enter_context(tc.tile_pool(name="o", bufs=4))

        ps_s = ctx.enter_context(tc.tile_pool(name="ps_s", bufs=2, space="PSUM"))
        ps_p = ctx.enter_context(tc.tile_pool(name="ps_p", bufs=1, space="PSUM"))
        ps_o = ctx.enter_context(tc.tile_pool(name="ps_o", bufs=5, space="PSUM"))

        # ---------------- resident SBUF tiles ----------------
        wp = const.tile([P, WP_N], BF16, tag="wp")
        wos = const.tile([65, D + 4], BF16, tag="wos")
        mu = const.tile([P, 2 * BAND], BF16, tag="mu")  # generated on device
        mg = const.tile([2, 128 + 2 * BAND], F32R, tag="mg")
        xq = const.tile([P, 8 * 1024], BF16, tag="xq")  # [p, (j, r)] j-chunk major
        # k and v share one tile: chunks 0..7 = k^T, 8..15 = v^T
        xkv = const.tile([P, 16 * S], BF16, tag="xkv")
        qh = const.tile([HD, 1024], F32R, tag="qh")
        kh = const.tile([HD, S], F32R, tag="kh")
        vh = const.tile([P, 16 * 65], BF16, tag="vh")  # [:, 65c:65c+64] + ones col

        # AV accumulators: w0 full bank; w1 quarters in their own banks
        # (one live accumulation chain per PSUM bank).
        pso_w0 = ps_o.tile([65, 512], F32, tag="po", name="pso_w0")
        pso_q = []
        for _qi in range(4):
            _pq = ps_o.tile([65, 512], F32, tag="po", name=f"pso_q{_qi}")
            pso_q.append(_pq)

        # ---------------- input DMAs (front-loaded, stream order) --------
        nc.gpsimd.memset(vh[:].rearrange("p (c e) -> p c e", e=65)[:, :, 64:65], 1.0)
        nc.scalar.dma_start(mg[:], _r(io["mgen"][:], F32R))
        nc.scalar.dma_start(wos[:], io["wos"][:])
        nc.sync.dma_start(wp[:], io["wp"][:])
        # w1's qh half first: the first k/v arrival is block 8 (w1-only)
        xq_s = xq[:].rearrange("p (j r) -> p j r", r=1024)
        xq_d = io["xqT"].rearrange("(j p) r -> p j r", p=P)
        nc.sync.dma_start(xq_s[:, :, 512:1024], xq_d[:, :, 512:1024])
        nc.sync.dma_start(xq_s[:, :, 0:512], xq_d[:, :, 0:512])
        # xkv columns are host-reordered into KV_ORDER so each arrival piece
        # (2 blocks = 1 MB) is contiguous with 512 B runs (no small-elem
        # penalty). SBUF keeps the reordered layout; _slot maps global block
        # id -> reordered slot.
        xkv_s = xkv[:].rearrange("p (j r) -> p j r", r=S)
        xkv_d = io["xkv"].rearrange("(j p) r -> p j r", p=P)
        for s in range(8):
            nc.sync.dma_start(
                xkv_s[:, :, 256 * s : 256 * s + 256],
                xkv_d[:, :, 256 * s : 256 * s + 256],
            )

        # ---------------- device-side causal masks ----------------------
        # mu[p, w*BAND + u] = 1.0 iff p <= u - X_w, via a rank-2 PE outer
        # product (iota difference) and a DVE >=0 compare. Runs in the
        # DMA-bound opening microseconds and warms up the PE p-state.
        for w in range(2):
            for c0, cw in ((0, 512), (512, 512), (1024, BAND - 1024)):
                psm = ps_s.tile([P, 512], F32, tag="ps")
                for rep in range(2):
                    nc.tensor.matmul(
                        psm[:, 0:cw],
                        mg[:, 0:128],
                        mg[:, 128 + BAND * w + c0 : 128 + BAND * w + c0 + cw],
                        start=True,
                        stop=True,
                    )
                nc.vector.tensor_scalar(
                    mu[:, BAND * w + c0 : BAND * w + c0 + cw],
                    psm[:, 0:cw],
                    0.0,
                    None,
                    mybir.AluOpType.is_ge,
                )

        # ---------------- projections ----------------
        def proj_q(w):
            psp_full = ps_p.tile([P, 512], F32, tag="pp")
            psp = psp_full[0:HD]
            for j in range(8):
                nc.tensor.matmul(
                    psp,
                    wp[:, WP_Q + 64 * j : WP_Q + 64 * j + 64],
                    xq[:, 1024 * j + 512 * w : 1024 * j + 512 * w + 512],
                    start=(j == 0),
                    stop=(j == 7),
                )
            nc.scalar.activation(
                qh[:, 512 * w : 512 * w + 512],
                psp,
                mybir.ActivationFunctionType.Identity,
                bias=_r(wos[0:HD, D : D + 2], F32),
                scale=1.0,
            )

        def proj_kv(c):
            """kh and vh for k/v block c (rows [128c, 128c+128))."""
            s = KV_ORDER.index(c)  # reordered SBUF slot
            psp_full = ps_p.tile([P, 512], F32, tag="pp")
            psk = psp_full[0:HD][:, 0:128]
            for j in range(8):
                nc.tensor.matmul(
                    psk,
                    wp[:, WP_K + 64 * j : WP_K + 64 * j + 64],
                    xkv[:, S * j + 128 * s : S * j + 128 * s + 128],
                    start=(j == 0),
                    stop=(j == 7),
                )
            nc.vector.tensor_scalar_add(
                kh[:, 128 * c : 128 * c + 128],
                psk,
                _r(wos[0:HD, D + 2 : D + 4], F32),
            )
            psv = psp_full[:, 256:320]
            for j in range(8):
                nc.tensor.matmul(
                    psv,
                    xkv[:, S * (8 + j) + 128 * s : S * (8 + j) + 128 * s + 128],
                    wp[:, WP_V + 64 * j : WP_V + 64 * j + 64],
                    start=(j == 0),
                    stop=(j == 7),
                )
            nc.vector.tensor_copy(vh[:, 65 * c : 65 * c + HD], psv)

        # ---------------- attention ----------------
        def attn_w(w, j):
            """Scores + exp (+ causal mask) of block j against window w.

            Returns the AV operand [128, 512] (cols = window q-columns;
            for trimmed deep blocks only the needed suffix is valid)."""
            if w == 0:
                lo = 0
            else:
                lo = 128 * max(0, j - 12)  # quarters k >= j-12 need block j
            width = 512 - lo
            pss = ps_s.tile([P, 512], F32, tag="ps")
            nc.tensor.matmul(
                pss[:, lo:512],
                kh[:, 128 * j : 128 * j + 128],
                qh[:, 512 * w + lo : 512 * w + 512],
                start=True,
                stop=True,
            )
            at = atp.tile([P, 512], BF16, tag="at")
            nc.scalar.activation(
                at[:, lo:512], pss[:, lo:512], mybir.ActivationFunctionType.Exp
            )
            # causal mask (multiplicative, on the exp'd tile). w0 bands
            # cover j<=7, w1 bands j>=8; other blocks are fully unmasked.
            toff = P * ((7 if w == 0 else 15) - j)
            if w == 0 or j >= 8:
                am = atp.tile([P, 512], BF16, tag="at")
                nc.vector.tensor_mul(
                    am[:, lo:512],
                    at[:, lo:512],
                    mu[:, BAND * w + toff + lo : BAND * w + toff + 512],
                )
                return am
            return at

        first = {"w0": True, "q": [True] * 4}

        def avs(j, src_w0, src_w1):
            if j < 8:
                nc.tensor.matmul(
                    pso_w0[:],
                    vh[:, 65 * j : 65 * j + 65],
                    src_w0[:, 0:512],
                    start=first["w0"],
                    stop=(j == W0_STOP),
                )
                first["w0"] = False
            for k in range(max(0, j - 12), 4):
                nc.tensor.matmul(
                    pso_q[k][:, 0:128],
                    vh[:, 65 * j : 65 * j + 65],
                    src_w1[:, 128 * k : 128 * k + 128],
                    start=first["q"][k],
                    stop=(j == Q_STOP[k]),
                )
                first["q"][k] = False

        def finish_w0():
            oh = ohp.tile([65, 512], BF16, tag="oh")
            nc.vector.tensor_copy(oh[:], pso_w0[:])
            nc.gpsimd.dma_start(io["esum"][0:1, :], oh[64:65, :])
            for t in range(4):
                ot = opool.tile([P, D], BF16, tag="o")
                pf = ps_s.tile([P, 512], F32, tag="ps")
                pf2 = ps_s.tile([P, 512], F32, tag="ps")
                nc.tensor.matmul(
                    pf, oh[:, 128 * t : 128 * t + 128], wos[:, 0:512],
                    start=True, stop=True,
                )
                nc.tensor.matmul(
                    pf2, oh[:, 128 * t : 128 * t + 128], wos[:, 512:1024],
                    start=True, stop=True,
                )
                nc.vector.tensor_copy(ot[:, 0:512], pf)
                nc.scalar.copy(ot[:, 512:1024], pf2)
                nc.sync.dma_start(
                    io["out"][128 * t : 128 * t + 128, :], ot[:]
                )

        def finish_q(k):
            oh = ohp.tile([65, 512], BF16, tag="oh")
            nc.vector.tensor_copy(oh[:, 0:128], pso_q[k][:, 0:128])
            nc.gpsimd.dma_start(
                io["esum"][1:2, 128 * k : 128 * k + 128], oh[64:65, 0:128]
            )
            ot = opool.tile([P, D], BF16, tag="o")
            pf = ps_s.tile([P, 512], F32, tag="ps")
            pf2 = ps_s.tile([P, 512], F32, tag="ps")
            nc.tensor.matmul(pf, oh[:, 0:128], wos[:, 0:512], start=True, stop=True)
            nc.tensor.matmul(
                pf2, oh[:, 0:128], wos[:, 512:1024], start=True, stop=True
            )
            if k % 2 == 0:
                nc.vector.tensor_copy(ot[:, 0:512], pf)
                nc.scalar.copy(ot[:, 512:1024], pf2)
            else:
                nc.scalar.copy(ot[:, 0:512], pf)
                nc.vector.tensor_copy(ot[:, 512:1024], pf2)
            nc.sync.dma_start(
                io["out"][512 + 128 * k : 512 + 128 * k + 128, :], ot[:]
            )

        # ---------------- schedule ----------------
        proj_q(1)
        proj_q(0)
        for c in KV_ORDER:
            proj_kv(c)
            a1 = attn_w(1, c)
            a0 = attn_w(0, c) if c < 8 else None
            avs(c, a0, a1)
            if c == W0_STOP:
                finish_w0()
                finish_q(0)
            for k in (1, 2, 3):
                if c == Q_STOP[k]:
                    finish_q(k)


_CACHE = {}


def _build():
    if "nc" in _CACHE:
        return _CACHE["nc"]
    nc = bacc.Bacc("TRN2", target_bir_lowering=False, debug=False, num_devices=NCORES)
    io = {}
    io["xqT"] = nc.dram_tensor("xqT", [D, 1024], BF16, kind="ExternalInput").ap()
    io["xkv"] = nc.dram_tensor("xkv", [2 * D, S], BF16, kind="ExternalInput").ap()
    io["wp"] = nc.dram_tensor("wp", [P, WP_N], BF16, kind="ExternalInput").ap()
    io["wos"] = nc.dram_tensor("wos", [65, D + 4], BF16, kind="ExternalInput").ap()
    io["mgen"] = nc.dram_tensor("mgen", [2, 128 + 2 * BAND], F32, kind="ExternalInput").ap()
    io["out"] = nc.dram_tensor("out", [1024, D], BF16, kind="ExternalOutput").ap()
    io["esum"] = nc.dram_tensor("esum", [2, 512], BF16, kind="ExternalOutput").ap()
    with tile.TileContext(nc) as tc:
        _emit(tc, io)
    nc.compile()
    _CACHE["nc"] = nc
    return nc


def _host_prep(query, key, value, mask, Wq, Wk, Wv, wq_h, bq_h, wk_h, bk_h, wv_h,
               bv_h, Wo):
    """Combine weights on host (exact algebra, float64 accumulate)."""
    Aq = (np.asarray(Wq, np.float64) @ np.asarray(wq_h, np.float64) / 8.0).astype(
        np.float32
    )
    Ak = (np.asarray(Wk, np.float64) @ np.asarray(wk_h, np.float64)).astype(np.float32)
    Av = (np.asarray(Wv, np.float64) @ np.asarray(wv_h, np.float64)).astype(np.float32)
    bq = (np.asarray(bq_h, np.float64) / 8.0).astype(np.float32)
    bk = np.asarray(bk_h, np.float32)
    WoS = np.asarray(Wo, np.float64).reshape(H, HD, D).sum(axis=0)
    wos_aug = np.concatenate(
        [WoS, (np.asarray(bv_h, np.float64) @ WoS)[None, :]], axis=0
    ).astype(np.float32)
    # biases ride as raw f32 bits in two bf16-pair columns:
    # cols D:D+2 = bq, cols D+2:D+4 = bk (device bitcasts back to f32)
    import ml_dtypes
    wos_ext = np.zeros((65, D + 4), ml_dtypes.bfloat16)
    wos_ext[:, 0:D] = wos_aug.astype(ml_dtypes.bfloat16)
    u16 = wos_ext.view(np.uint16)
    u16[0:HD, D : D + 2] = bq.astype(np.float32).view(np.uint16).reshape(HD, 2)
    u16[0:HD, D + 2 : D + 4] = bk.astype(np.float32).view(np.uint16).reshape(HD, 2)
    return Aq, Ak, Av, wos_ext


def _pack_w(A):
    """[1024, 64] -> [128, 512] partition-packed layout."""
    return np.ascontiguousarray(
        A.reshape(8, P, HD).transpose(1, 0, 2).reshape(P, 512)
    )


def _mk_mgen(h):
    """Inputs for device-side mask generation.

    Row 0: [ones(128) | u - X_0 | u - X_1]; row 1: [iota(128) | -1 | -1].
    The PE computes psm[p, u] = (u - X_w) - p; keep iff >= 0.
    """
    u = np.arange(BAND, dtype=np.float32)
    mgen = np.empty((2, 128 + 2 * BAND), np.float32)
    mgen[0, 0:128] = 1.0
    mgen[1, 0:128] = np.arange(128, dtype=np.float32)
    mgen[1, 128:] = -1.0
    mgen[0, 128 : 128 + BAND] = u - np.float32(896 - _q0(h, 0))
    mgen[0, 128 + BAND :] = u - np.float32(1920 - _q0(h, 1))
    return mgen


def _numpy_fallback(query, key, value, mask, Wq, Wk, Wv, wq_h, bq_h, wk_h, bk_h,
                    wv_h, bv_h, Wo):
    q = query @ Wq
    k = key @ Wk
    v = value @ Wv
    qh = q @ wq_h + bq_h
    kh = k @ wk_h + bk_h
    vh = v @ wv_h + bv_h
    scores = np.einsum("bsh,bth->bst", qh, kh) / np.sqrt(np.float32(HD))
    scores = np.where(mask, np.float32(-1e9), scores)
    scores = scores - scores.max(axis=-1, keepdims=True)
    e = np.exp(scores)
    attn = e / e.sum(axis=-1, keepdims=True)
    out_h = np.einsum("bst,bth->bsh", attn, vh)
    out = np.tile(out_h, (1, 1, H))
    return (out @ Wo).astype(np.float32)


def kernel(**inputs):
    import ml_dtypes

    inputs = {k: np.asarray(v) for k, v in inputs.items()}
    mask = inputs["mask"]
    causal = np.array_equal(mask, np.triu(np.ones((S, S), bool), k=1))
    if not causal:
        return _numpy_fallback(**inputs)

    query, key, value = inputs["query"], inputs["key"], inputs["value"]
    Aq, Ak, Av, wos_ext = _host_prep(**inputs)

    wp = np.zeros((P, WP_N), ml_dtypes.bfloat16)
    wp[:, WP_Q : WP_Q + 512] = _pack_w(Aq).astype(ml_dtypes.bfloat16)
    wp[:, WP_K : WP_K + 512] = _pack_w(Ak).astype(ml_dtypes.bfloat16)
    wp[:, WP_V : WP_V + 512] = _pack_w(Av).astype(ml_dtypes.bfloat16)

    nc = _build()
    xkv = {}
    for b in range(B):
        kT = key[b].T.astype(ml_dtypes.bfloat16)
        vT = value[b].T.astype(ml_dtypes.bfloat16)
        buf = np.empty((2 * D, S), ml_dtypes.bfloat16)
        # columns reordered into arrival order (KV_ORDER): slot s holds
        # global 128-row block KV_ORDER[s]
        for s, c in enumerate(KV_ORDER):
            buf[0:D, 128 * s : 128 * s + 128] = kT[:, 128 * c : 128 * c + 128]
            buf[D:, 128 * s : 128 * s + 128] = vT[:, 128 * c : 128 * c + 128]
        xkv[b] = buf
    in_maps = []
    for c in range(NCORES):
        b, h = c // 2, c % 2
        xq_rows = np.concatenate(
            [
                query[b, _q0(h, 0) : _q0(h, 0) + 512],
                query[b, _q0(h, 1) : _q0(h, 1) + 512],
            ],
            axis=0,
        )
        in_maps.append(
            {
                "xqT": np.ascontiguousarray(xq_rows.T.astype(ml_dtypes.bfloat16)),
                "xkv": xkv[b],
                "wp": wp,
                "wos": wos_ext,
                "mgen": _mk_mgen(h),
            }
        )

    res = run_bass_kernel_spmd(nc, in_maps, list(range(NCORES)))
    out = np.empty((B, S, D), np.float32)
    for c in range(NCORES):
        b, h = c // 2, c % 2
        co = np.asarray(res.results[c]["out"]).astype(np.float32)
        es = np.asarray(res.results[c]["esum"]).astype(np.float32)
        co[0:512] /= es[0][:, None]
        co[512:1024] /= es[1][:, None]
        out[b, _q0(h, 0) : _q0(h, 0) + 512] = co[0:512]
        out[b, _q0(h, 1) : _q0(h, 1) + 512] = co[512:1024]
    return out


if __name__ == "__main__":
    nc = _build()
    print("build ok")


# revision 16
# speedup vs baseline: 1.3844x; 1.0862x over previous
"""Trainium2 Bass kernel for nn_MultiHeadAttention_56676388438432 (v4).

Reference math (all H=16 heads share identical weights, so they collapse):
    q = query @ Wq; k = key @ Wk; v = value @ Wv          (full-width, [B,S,D])
    qh = q @ wq_h + bq_h                                   ([B,S,64])
    scores = qh @ kh^T / 8, causal mask, softmax
    out_h = attn @ vh
    out = tile(out_h, 16) @ Wo

Algebraic collapse (exact):
    qh = query @ (Wq @ wq_h / 8) + bq_h/8   -> combined Aq [1024, 64]
    kh = key   @ (Wk @ wk_h) + bk_h         -> combined Ak
    vh = value @ (Wv @ wv_h)        (bv_h folded into Wo, see below)
    out = [out_h ; expsum] @ [WoS ; bv_h @ WoS]   (WoS = sum_h Wo[64h:64h+64])

The kernel is DMA-bound: 12.5 MB/core through an exclusive 360 B/ns DMA
device = 34.7 us floor. v4 restructures v2's schedule so the device never
idles and the post-stream tail is one 128-row chain instead of a 512-row
window:
  * window 1 (the deep 512 rows) is split into four 128-col AV chains in
    separate PSUM banks with staggered stop blocks (12, 13, 14, 15 at
    h=0 depth); their finishes and output DMAs spread through the
    stream instead of bunching after the last k/v byte.
  * k/v arrive one 128-row block at a time, ordered
    [8,0,9,1,10,2,11,3,12,4,5,6,7,13,14,15]: deep/shallow interleave
    balances per-arrival compute, every chain except the deepest quarter
    completes mid-stream, and the last three arrivals carry minimal work.
  * all DMAs ride the two HWDGE queues (SP + Act); outputs/esums go on
    Act, inputs on SP (the Pool SWDGE path costs ~2.4x per transfer).
  * scores are unpaired [128, <=512] per (window, block) so score PSUM
    tiles are single banks; exp widths shrink for blocks 13-15.

Sharding (unchanged from v2): 8 cores = 4 batches x 2 balanced causal
halves; w0 = rows [512h, 512h+512), w1 = rows [1536-512h, 2048-512h).
Each core redundantly computes kh/vh for its batch. Uniform SPMD
program; softmax division by expsum happens on host.
"""

import sys

sys.path.insert(0, "/opt/trn_rl_repo")

from contextlib import ExitStack

import numpy as np

import concourse.bass as bass
import concourse.tile as tile
from concourse import bacc
from concourse import mybir
from concourse.bass_utils import run_bass_kernel_spmd

B, S, D, H, HD = 4, 2048, 1024, 16, 64
P = 128
NCORES = 8

F32 = mybir.dt.float32
F32R = mybir.dt.float32r
BF16 = mybir.dt.bfloat16

W_SLOTS = (8, 16)
BAND = 1408  # mask band width: 128*7 + 512

WP_Q, WP_K, WP_V = 0, 512, 1024
WP_N = 1536

# k/v block arrival order: shallow/deep interleave; blocks 13-15 last so
# only the deepest quarter-chain's work trails the stream.
KV_ORDER = [8, 0, 9, 1, 10, 2, 11, 3, 12, 4, 5, 6, 7, 13, 14, 15]
# w1 is split into three AV chains: q01 = cols 0:256 (rows +0:256 of the
# window, stops when {0..13} is exhausted = block 13), q2 = cols 256:384
# (stop 14), q3 = cols 384:512 (stop 15). w0 stops at block 7 (the last
# of {0..7} in KV_ORDER). Chain trims align with chain boundaries so no
# accumulation instruction ever writes a partial region of its chain.
W0_STOP = 7


def _q0(h, w):
    return 512 * h if w == 0 else 1536 - 512 * h


def _r(ap, dt):
    return ap.bitcast(dt)


def _emit(tc, io):
    """Emit the per-core program. io: dict of DRAM APs."""
    nc = tc.nc
    ctx = ExitStack()
    with ctx:
        # ---------------- pools ----------------
        const = ctx.enter_context(tc.tile_pool(name="const", bufs=1))
        atp = ctx.enter_context(tc.tile_pool(name="attn", bufs=6))
        ohp = ctx.enter_context(tc.tile_pool(name="oh", bufs=3))
        opool = ctx.enter_context(tc.tile_pool(name="o", bufs=4))

        ps_s = ctx.enter_context(tc.tile_pool(name="ps_s", bufs=2, space="PSUM"))
        ps_p = ctx.enter_context(tc.tile_pool(name="ps_p", bufs=2, space="PSUM"))
        ps_o = ctx.enter_context(tc.tile_pool(name="ps_o", bufs=4, space="PSUM"))

        # ---------------- resident SBUF tiles ----------------
        wp = const.tile([P, WP_N], BF16, tag="wp")
        wos = const.tile([65, D + 4], BF16, tag="wos")
        mu = const.tile([P, 2 * BAND], BF16, tag="mu")  # generated on device
        mg = const.tile([2, 128 + 2 * BAND], F32R, tag="mg")
        xq = const.tile([P, 8 * 1024], BF16, tag="xq")  # [p, (j, r)] j-chunk major
        # k and v share one tile: chunks 0..7 = k^T, 8..15 = v^T
        xkv = const.tile([P, 16 * S], BF16, tag="xkv")
        qh = const.tile([HD, 1024], F32R, tag="qh")
        kh = const.tile([HD, S], F32R, tag="kh")
        vh = const.tile([P, 16 * 65], BF16, tag="vh")  # [:, 65c:65c+64] + ones col

        # AV accumulators, one live chain per PSUM bank: w0 full bank;
        # w1 split q01 (256 cols) / q2 (128) / q3 (128).
        pso_w0 = ps_o.tile([65, 512], F32, tag="po", name="pso_w0")
        pso_q01 = ps_o.tile([65, 512], F32, tag="po", name="pso_q01")
        pso_q2 = ps_o.tile([65, 512], F32, tag="po", name="pso_q2")
        pso_q3 = ps_o.tile([65, 512], F32, tag="po", name="pso_q3")

        # ---------------- input DMAs (front-loaded, stream order) --------
        nc.gpsimd.memset(vh[:].rearrange("p (c e) -> p c e", e=65)[:, :, 64:65], 1.0)
        nc.scalar.dma_start(mg[:], _r(io["mgen"][:], F32R))
        nc.scalar.dma_start(wos[:], io["wos"][:])
        nc.sync.dma_start(wp[:], io["wp"][:])
        # w1's qh half first: the first k/v arrival is block 8 (w1-only)
        xq_s = xq[:].rearrange("p (j r) -> p j r", r=1024)
        xq_d = io["xqT"].rearrange("(j p) r -> p j r", p=P)
        nc.sync.dma_start(xq_s[:, :, 512:1024], xq_d[:, :, 512:1024])
        nc.sync.dma_start(xq_s[:, :, 0:512], xq_d[:, :, 0:512])
        # xkv columns are host-reordered into KV_ORDER so each arrival piece
        # (2 blocks = 1 MB) is contiguous with 512 B runs (no small-elem
        # penalty). SBUF keeps the reordered layout; _slot maps global block
        # id -> reordered slot.
        xkv_s = xkv[:].rearrange("p (j r) -> p j r", r=S)
        xkv_d = io["xkv"].rearrange("(j p) r -> p j r", p=P)
        for s in range(8):
            nc.sync.dma_start(
                xkv_s[:, :, 256 * s : 256 * s + 256],
                xkv_d[:, :, 256 * s : 256 * s + 256],
            )

        # ---------------- device-side causal masks ----------------------
        # mu[p, w*BAND + u] = 1.0 iff p <= u - X_w, via a rank-2 PE outer
        # product (iota difference) and a DVE >=0 compare. Runs in the
        # DMA-bound opening microseconds and warms up the PE p-state.
        for w in range(2):
            for c0, cw in ((0, 512), (512, 512), (1024, BAND - 1024)):
                psm = ps_s.tile([P, 512], F32, tag="ps")
                for rep in range(2):
                    nc.tensor.matmul(
                        psm[:, 0:cw],
                        mg[:, 0:128],
                        mg[:, 128 + BAND * w + c0 : 128 + BAND * w + c0 + cw],
                        start=True,
                        stop=True,
                    )
                nc.vector.tensor_scalar(
                    mu[:, BAND * w + c0 : BAND * w + c0 + cw],
                    psm[:, 0:cw],
                    0.0,
                    None,
                    mybir.AluOpType.is_ge,
                )

        # ---------------- projections ----------------
        def proj_q(w):
            psp_full = ps_p.tile([P, 512], F32, tag="pp")
            psp = psp_full[0:HD]
            for j in range(8):
                nc.tensor.matmul(
                    psp,
                    wp[:, WP_Q + 64 * j : WP_Q + 64 * j + 64],
                    xq[:, 1024 * j + 512 * w : 1024 * j + 512 * w + 512],
                    start=(j == 0),
                    stop=(j == 7),
                )
            nc.scalar.activation(
                qh[:, 512 * w : 512 * w + 512],
                psp,
                mybir.ActivationFunctionType.Identity,
                bias=_r(wos[0:HD, D : D + 2], F32),
                scale=1.0,
            )

        def proj_kv(c):
            """kh and vh for k/v block c (rows [128c, 128c+128))."""
            s = KV_ORDER.index(c)  # reordered SBUF slot
            psp_full = ps_p.tile([P, 512], F32, tag="pp")
            psk = psp_full[0:HD][:, 0:128]
            for j in range(8):
                nc.tensor.matmul(
                    psk,
                    wp[:, WP_K + 64 * j : WP_K + 64 * j + 64],
                    xkv[:, S * j + 128 * s : S * j + 128 * s + 128],
                    start=(j == 0),
                    stop=(j == 7),
                )
            nc.vector.tensor_scalar_add(
                kh[:, 128 * c : 128 * c + 128],
                psk,
                _r(wos[0:HD, D + 2 : D + 4], F32),
            )
            psv = psp_full[:, 256:320]
            for j in range(8):
                nc.tensor.matmul(
                    psv,
                    xkv[:, S * (8 + j) + 128 * s : S * (8 + j) + 128 * s + 128],
                    wp[:, WP_V + 64 * j : WP_V + 64 * j + 64],
                    start=(j == 0),
                    stop=(j == 7),
                )
            nc.vector.tensor_copy(vh[:, 65 * c : 65 * c + HD], psv)

        # ---------------- attention ----------------
        def attn_w(w, j):
            """Scores + exp (+ causal mask) of block j against window w.

            Returns the AV operand [128, 512] (cols = window q-columns;
            for trimmed deep blocks only the needed suffix is valid)."""
            if w == 0 or j <= 13:
                lo = 0
            else:
                lo = 256 if j == 14 else 384  # chains ending before j skip
            width = 512 - lo
            pss = ps_s.tile([P, 512], F32, tag="ps")
            nc.tensor.matmul(
                pss[:, lo:512],
                kh[:, 128 * j : 128 * j + 128],
                qh[:, 512 * w + lo : 512 * w + 512],
                start=True,
                stop=True,
            )
            at = atp.tile([P, 512], BF16, tag="at")
            nc.scalar.activation(
                at[:, lo:512], pss[:, lo:512], mybir.ActivationFunctionType.Exp
            )
            # causal mask (multiplicative, on the exp'd tile). w0 bands
            # cover j<=7, w1 bands j>=8; other blocks are fully unmasked.
            toff = P * ((7 if w == 0 else 15) - j)
            if w == 0 or j >= 8:
                am = atp.tile([P, 512], BF16, tag="at")
                nc.vector.tensor_mul(
                    am[:, lo:512],
                    at[:, lo:512],
                    mu[:, BAND * w + toff + lo : BAND * w + toff + 512],
                )
                return am
            return at

        first = {"w0": True, "q01": True, "q2": True, "q3": True}

        def avs(j, src_w0, src_w1):
            vhj = vh[:, 65 * j : 65 * j + 65]
            if j < 8:
                nc.tensor.matmul(
                    pso_w0[:], vhj, src_w0[:, 0:512],
                    start=first["w0"], stop=(j == W0_STOP),
                )
                first["w0"] = False
            if j <= 13:
                nc.tensor.matmul(
                    pso_q01[:, 0:256], vhj, src_w1[:, 0:256],
                    start=first["q01"], stop=(j == 13),
                )
                first["q01"] = False
            if j <= 14:
                nc.tensor.matmul(
                    pso_q2[:, 0:128], vhj, src_w1[:, 256:384],
                    start=first["q2"], stop=(j == 14),
                )
                first["q2"] = False
            nc.tensor.matmul(
                pso_q3[:, 0:128], vhj, src_w1[:, 384:512],
                start=first["q3"], stop=(j == 15),
            )
            first["q3"] = False

        def finish_w0():
            oh = ohp.tile([65, 512], BF16, tag="oh")
            nc.vector.tensor_copy(oh[:], pso_w0[:])
            nc.sync.dma_start(io["esum"][0:1, :], oh[64:65, :])
            for t in range(4):
                ot = opool.tile([P, D], BF16, tag="o")
                pf = ps_s.tile([P, 512], F32, tag="ps")
                pf2 = ps_s.tile([P, 512], F32, tag="ps")
                nc.tensor.matmul(
                    pf, oh[:, 128 * t : 128 * t + 128], wos[:, 0:512],
                    start=True, stop=True,
                )
                nc.tensor.matmul(
                    pf2, oh[:, 128 * t : 128 * t + 128], wos[:, 512:1024],
                    start=True, stop=True,
                )
                nc.vector.tensor_copy(ot[:, 0:512], pf)
                nc.scalar.copy(ot[:, 512:1024], pf2)
                nc.sync.dma_start(
                    io["out"][128 * t : 128 * t + 128, :], ot[:]
                )

        def finish_q(pso, c0, cw):
            """Finish w1 columns [c0, c0+cw) (out rows 512+c0 ..)."""
            oh = ohp.tile([65, 512], BF16, tag="oh")
            nc.vector.tensor_copy(oh[:, 0:cw], pso[:, 0:cw])
            nc.sync.dma_start(
                io["esum"][1:2, c0 : c0 + cw], oh[64:65, 0:cw]
            )
            for t in range(cw // 128):
                ot = opool.tile([P, D], BF16, tag="o")
                pf = ps_s.tile([P, 512], F32, tag="ps")
                pf2 = ps_s.tile([P, 512], F32, tag="ps")
                nc.tensor.matmul(
                    pf, oh[:, 128 * t : 128 * t + 128], wos[:, 0:512],
                    start=True, stop=True,
                )
                nc.tensor.matmul(
                    pf2, oh[:, 128 * t : 128 * t + 128], wos[:, 512:1024],
                    start=True, stop=True,
                )
                if t % 2 == 0:
                    nc.vector.tensor_copy(ot[:, 0:512], pf)
                    nc.scalar.copy(ot[:, 512:1024], pf2)
                else:
                    nc.scalar.copy(ot[:, 0:512], pf)
                    nc.vector.tensor_copy(ot[:, 512:1024], pf2)
                nc.sync.dma_start(
                    io["out"][512 + c0 + 128 * t : 512 + c0 + 128 * t + 128, :],
                    ot[:],
                )

        # ---------------- schedule ----------------
        proj_q(1)
        proj_q(0)
        for c in KV_ORDER:
            proj_kv(c)
            a1 = attn_w(1, c)
            a0 = attn_w(0, c) if c < 8 else None
            avs(c, a0, a1)
            if c == W0_STOP:
                finish_w0()
            elif c == 13:
                finish_q(pso_q01, 0, 256)
            elif c == 14:
                finish_q(pso_q2, 256, 128)
            elif c == 15:
                finish_q(pso_q3, 384, 128)


_CACHE = {}


def _build():
    if "nc" in _CACHE:
        return _CACHE["nc"]
    nc = bacc.Bacc("TRN2", target_bir_lowering=False, debug=False, num_devices=NCORES)
    io = {}
    io["xqT"] = nc.dram_tensor("xqT", [D, 1024], BF16, kind="ExternalInput").ap()
    io["xkv"] = nc.dram_tensor("xkv", [2 * D, S], BF16, kind="ExternalInput").ap()
    io["wp"] = nc.dram_tensor("wp", [P, WP_N], BF16, kind="ExternalInput").ap()
    io["wos"] = nc.dram_tensor("wos", [65, D + 4], BF16, kind="ExternalInput").ap()
    io["mgen"] = nc.dram_tensor("mgen", [2, 128 + 2 * BAND], F32, kind="ExternalInput").ap()
    io["out"] = nc.dram_tensor("out", [1024, D], BF16, kind="ExternalOutput").ap()
    io["esum"] = nc.dram_tensor("esum", [2, 512], BF16, kind="ExternalOutput").ap()
    with tile.TileContext(nc) as tc:
        _emit(tc, io)
    nc.compile()
    _CACHE["nc"] = nc
    return nc


def _host_prep(query, key, value, mask, Wq, Wk, Wv, wq_h, bq_h, wk_h, bk_h, wv_h,
               bv_h, Wo):
    """Combine weights on host (exact algebra, float64 accumulate)."""
    Aq = (np.asarray(Wq, np.float64) @ np.asarray(wq_h, np.float64) / 8.0).astype(
        np.float32
    )
    Ak = (np.asarray(Wk, np.float64) @ np.asarray(wk_h, np.float64)).astype(np.float32)
    Av = (np.asarray(Wv, np.float64) @ np.asarray(wv_h, np.float64)).astype(np.float32)
    bq = (np.asarray(bq_h, np.float64) / 8.0).astype(np.float32)
    bk = np.asarray(bk_h, np.float32)
    WoS = np.asarray(Wo, np.float64).reshape(H, HD, D).sum(axis=0)
    wos_aug = np.concatenate(
        [WoS, (np.asarray(bv_h, np.float64) @ WoS)[None, :]], axis=0
    ).astype(np.float32)
    # biases ride as raw f32 bits in two bf16-pair columns:
    # cols D:D+2 = bq, cols D+2:D+4 = bk (device bitcasts back to f32)
    import ml_dtypes
    wos_ext = np.zeros((65, D + 4), ml_dtypes.bfloat16)
    wos_ext[:, 0:D] = wos_aug.astype(ml_dtypes.bfloat16)
    u16 = wos_ext.view(np.uint16)
    u16[0:HD, D : D + 2] = bq.astype(np.float32).view(np.uint16).reshape(HD, 2)
    u16[0:HD, D + 2 : D + 4] = bk.astype(np.float32).view(np.uint16).reshape(HD, 2)
    return Aq, Ak, Av, wos_ext


def _pack_w(A):
    """[1024, 64] -> [128, 512] partition-packed layout."""
    return np.ascontiguousarray(
        A.reshape(8, P, HD).transpose(1, 0, 2).reshape(P, 512)
    )


def _mk_mgen(h):
    """Inputs for device-side mask generation.

    Row 0: [ones(128) | u - X_0 | u - X_1]; row 1: [iota(128) | -1 | -1].
    The PE computes psm[p, u] = (u - X_w) - p; keep iff >= 0.
    """
    u = np.arange(BAND, dtype=np.float32)
    mgen = np.empty((2, 128 + 2 * BAND), np.float32)
    mgen[0, 0:128] = 1.0
    mgen[1, 0:128] = np.arange(128, dtype=np.float32)
    mgen[1, 128:] = -1.0
    mgen[0, 128 : 128 + BAND] = u - np.float32(896 - _q0(h, 0))
    mgen[0, 128 + BAND :] = u - np.float32(1920 - _q0(h, 1))
    return mgen


def _numpy_fallback(query, key, value, mask, Wq, Wk, Wv, wq_h, bq_h, wk_h, bk_h,
                    wv_h, bv_h, Wo):
    q = query @ Wq
    k = key @ Wk
    v = value @ Wv
    qh = q @ wq_h + bq_h
    kh = k @ wk_h + bk_h
    vh = v @ wv_h + bv_h
    scores = np.einsum("bsh,bth->bst", qh, kh) / np.sqrt(np.float32(HD))
    scores = np.where(mask, np.float32(-1e9), scores)
    scores = scores - scores.max(axis=-1, keepdims=True)
    e = np.exp(scores)
    attn = e / e.sum(axis=-1, keepdims=True)
    out_h = np.einsum("bst,bth->bsh", attn, vh)
    out = np.tile(out_h, (1, 1, H))
    return (out @ Wo).astype(np.float32)


def kernel(**inputs):
    import ml_dtypes

    inputs = {k: np.asarray(v) for k, v in inputs.items()}
    mask = inputs["mask"]
    causal = np.array_equal(mask, np.triu(np.ones((S, S), bool), k=1))
    if not causal:
        return _numpy_fallback(**inputs)

    query, key, value = inputs["query"], inputs["key"], inputs["value"]
    Aq, Ak, Av, wos_ext = _host_prep(**inputs)

    wp = np.zeros((P, WP_N), ml_dtypes.bfloat16)
    wp[:, WP_Q : WP_Q + 512] = _pack_w(Aq).astype(ml_dtypes.bfloat16)
    wp[:, WP_K : WP_K + 512] = _pack_w(Ak).astype(ml_dtypes.bfloat16)
    wp[:, WP_V : WP_V + 512] = _pack_w(Av).astype(ml_dtypes.bfloat16)

    nc = _build()
    xkv = {}
    for b in range(B):
        kT = key[b].T.astype(ml_dtypes.bfloat16)
        vT = value[b].T.astype(ml_dtypes.bfloat16)
        buf = np.empty((2 * D, S), ml_dtypes.bfloat16)
        # columns reordered into arrival order (KV_ORDER): slot s holds
        # global 128-row block KV_ORDER[s]
        for s, c in enumerate(KV_ORDER):
            buf[0:D, 128 * s : 128 * s + 128] = kT[:, 128 * c : 128 * c + 128]
            buf[D:, 128 * s : 128 * s + 128] = vT[:, 128 * c : 128 * c + 128]
        xkv[b] = buf
    in_maps = []
    for c in range(NCORES):
        b, h = c // 2, c % 2
        xq_rows = np.concatenate(
            [
                query[b, _q0(h, 0) : _q0(h, 0) + 512],
                query[b, _q0(h, 1) : _q0(h, 1) + 512],
            ],
            axis=0,
        )
        in_maps.append(
            {
                "xqT": np.ascontiguousarray(xq_rows.T.astype(ml_dtypes.bfloat16)),
                "xkv": xkv[b],
                "wp": wp,
                "wos": wos_ext,
                "mgen": _mk_mgen(h),
            }
        )

    res = run_bass_kernel_spmd(nc, in_maps, list(range(NCORES)))
    out = np.empty((B, S, D), np.float32)
    for c in range(NCORES):
        b, h = c // 2, c % 2
        co = np.asarray(res.results[c]["out"]).astype(np.float32)
        es = np.asarray(res.results[c]["esum"]).astype(np.float32)
        co[0:512] /= es[0][:, None]
        co[512:1024] /= es[1][:, None]
        out[b, _q0(h, 0) : _q0(h, 0) + 512] = co[0:512]
        out[b, _q0(h, 1) : _q0(h, 1) + 512] = co[512:1024]
    return out


if __name__ == "__main__":
    nc = _build()
    print("build ok")
